# revision 1
# baseline (speedup 1.0000x reference)
"""Trainium2 Bass kernel for 7x7 valid cross-correlation on a 4096x4096 image.

Strategy: shard output rows across 8 NeuronCores (512 rows/core). Each core
receives its input row-slab WITH the (kh-1)=6 halo rows already included, so
no device-side halo exchange is needed. On-core, the conv is computed on the
tensor engine as 7 PSUM-accumulating matmuls per output tile: for each kernel
column dx, a banded-Toeplitz stationary matrix B_dx[k, m] = w[k-m, dx]
contracts over 128 input rows while the moving operand is a column-shifted
view X[:, c0+dx : c0+dx+N] of the input slab already in SBUF.

Every matmul is the same verified shape (K=128, M=122, N=512, fp32r): the
last row/column tiles overlap their predecessors and only the fresh rows
are written out (fp32r gives wrong results for partial K or odd N).
"""

import numpy as np

H, W = 4096, 4096
KH, KW = 7, 7
N_CORES = 8
OH, OW = H - KH + 1, W - KW + 1          # 4090, 4090
RPC = H // N_CORES                        # 512 output rows per core (core 7: 506 valid)
IN_ROWS = RPC + KH - 1                    # 518 input rows per core
MT = 122                                  # output rows per row tile (128 - 6)
# (input/output row offset within slab, rows of outt to emit: [emit0, 122))
ROW_TILES = [(0, 0), (122, 0), (244, 0), (366, 0), (390, 98)]
CT_N = 512
# column tile output starts; last overlaps so every matmul has N=512
COL_STARTS = [0, 512, 1024, 1536, 2048, 2560, 3072, 3578]

# fp32r = relaxed-precision fp32 matmul (TF32-like): 1 cycle/row vs 4 for fp32.
USE_FP32R = True

_cache = {}


def _build_program(repeat=1):
    import concourse.bacc as bacc
    import concourse.mybir as mybir
    import concourse.tile as tile

    mm_dt = mybir.dt.float32r if USE_FP32R else mybir.dt.float32
    f32 = mybir.dt.float32

    nc = bacc.Bacc("TRN2", target_bir_lowering=False, debug=False,
                   num_devices=N_CORES)
    x = nc.dram_tensor("x", [IN_ROWS, W], mm_dt, kind="ExternalInput")
    bands = nc.dram_tensor("bands", [128, KW, MT], mm_dt, kind="ExternalInput")
    biasb = nc.dram_tensor("biasb", [128, 1], f32, kind="ExternalInput")
    y = nc.dram_tensor("y", [RPC, OW], f32, kind="ExternalOutput")

    with tile.TileContext(nc) as tc:
        with (
            tc.tile_pool(name="const", bufs=1) as cpool,
            tc.tile_pool(name="xs", bufs=3) as xpool,
            tc.tile_pool(name="out", bufs=2) as opool,
            tc.tile_pool(name="ps", bufs=8, space="PSUM") as pspool,
        ):
            bands_t = cpool.tile([128, KW, MT], mm_dt)
            nc.sync.dma_start(bands_t[:], bands[:])
            bias_t = cpool.tile([128, 1], f32)
            nc.sync.dma_start(bias_t[:], biasb[:])

            # Slabs are DMAed in column chunks: the first column tiles'
            # matmuls start before the whole 2MB slab lands, and chunks
            # pipeline better with compute than one monolithic DMA.
            first_chunks = [(0, 1030), (1024, 1030), (2048, 1030),
                            (3072, 1024)]

            for rep in range(repeat):
                for it, (r0, emit0) in enumerate(ROW_TILES):
                    xs = xpool.tile([128, W], mm_dt, tag="xs")
                    chunks = first_chunks
                    for cc0, cw in chunks:
                        # scalar-engine HWDGE ring: runs parallel to the
                        # const/output DMAs on the sync-engine ring
                        nc.scalar.dma_start(xs[:, cc0:cc0 + cw],
                                            x[r0:r0 + 128, cc0:cc0 + cw])
                    outt = opool.tile([128, OW], f32, tag="out")
                    for c0 in COL_STARTS:
                        ps = pspool.tile([128, CT_N], f32, tag="ps")
                        for dx in range(KW):
                            nc.tensor.matmul(
                                ps[:MT, :],
                                bands_t[:, dx, :],
                                xs[:, c0 + dx: c0 + dx + CT_N],
                                start=(dx == 0),
                                stop=(dx == KW - 1),
                            )
                        nc.vector.tensor_scalar_add(
                            outt[:MT, c0:c0 + CT_N], ps[:MT, :],
                            bias_t[:MT, 0:1])
                    nc.sync.dma_start(
                        y[r0 + emit0: r0 + MT, :], outt[emit0:MT, :])

    nc.compile()
    return nc


def _get_program():
    if "nc" not in _cache:
        _cache["nc"] = _build_program()
    return _cache["nc"]


def _shard_inputs(X, weight, bias):
    X = np.ascontiguousarray(np.asarray(X, dtype=np.float32))
    weight = np.asarray(weight, dtype=np.float32)
    bias = np.asarray(bias, dtype=np.float32)

    # Host-side sharding: per-core input slab with halo rows (zero-padded at
    # the bottom edge for the last core; those output rows are discarded).
    slabs = np.zeros((N_CORES, IN_ROWS, W), dtype=np.float32)
    for i in range(N_CORES):
        r0 = RPC * i
        r1 = min(r0 + IN_ROWS, H)
        slabs[i, : r1 - r0] = X[r0:r1]

    # Banded-Toeplitz stationary matrices: bands[k, dx, m] = w[k-m, dx].
    bands = np.zeros((128, KW, MT), dtype=np.float32)
    for dy in range(KH):
        for m in range(MT):
            bands[m + dy, :, m] = weight[dy, :]

    biasb = np.broadcast_to(bias.reshape(1, 1), (128, 1)).copy()

    return [{"x": slabs[i], "bands": bands, "biasb": biasb}
            for i in range(N_CORES)]


def kernel(X, weight, bias):
    from concourse.bass_utils import run_bass_kernel_spmd

    nc = _get_program()
    in_maps = _shard_inputs(X, weight, bias)
    res = run_bass_kernel_spmd(nc, in_maps, list(range(N_CORES)))

    out = np.empty((OH, OW), dtype=np.float32)
    for i in range(N_CORES):
        r0 = RPC * i
        nrows = min(RPC, OH - r0)
        out[r0:r0 + nrows] = res.results[i]["y"][:nrows]
    return out



# revision 2
# speedup vs baseline: 1.0532x; 1.0532x over previous
"""Trainium2 Bass kernel for 7x7 valid cross-correlation on a 4096x4096 image.

Strategy: shard output COLUMNS across 8 NeuronCores (512 cols/core + 6 halo
input cols). PE matmul cost is proportional to the streamed free dim (width)
and independent of the output-row count M, so the per-core work is the number
of 122-row band passes times the strip width. Column strips let every core
run the global minimum ceil(4090/122)=34 band passes over a 1/8-width strip
(34 * 7 * 512 cycles ~= 50.8us), vs 5 full-width passes (59.7us) for row
sharding.

On-core, each band pass is 7 PSUM-accumulating matmuls: for kernel column dx,
a banded-Toeplitz stationary matrix B_dx[k, m] = w[k-m, dx] contracts over
128 input rows while the moving operand is the column-shifted view
xs[:, dx:dx+512] of the strip tile in SBUF. Every matmul is the same
verified shape (K=128, M=122, N=512, fp32r); the last band pass overlaps its
predecessor and only emits the fresh rows (fp32r gives wrong results for
partial K or odd N).
"""

import numpy as np

H, W = 4096, 4096
KH, KW = 7, 7
N_CORES = 8
OH, OW = H - KH + 1, W - KW + 1          # 4090, 4090
CPC = 512                                 # output cols per core (core 7: 506 valid)
IN_COLS = CPC + KW - 1                    # 518 input cols per core (zero-padded for core 7)
MT = 122                                  # output rows per band pass (128 - 6)
# (input row offset, first fresh output row within the pass)
ROW_TILES = [(122 * t, 0) for t in range(33)] + [(H - 128, 58)]

# fp32r = relaxed-precision fp32 matmul (TF32-like): 1 cycle/row vs 4 for fp32.
USE_FP32R = True

_cache = {}


def _build_program(repeat=1):
    import concourse.bacc as bacc
    import concourse.mybir as mybir
    import concourse.tile as tile

    mm_dt = mybir.dt.float32r if USE_FP32R else mybir.dt.float32
    f32 = mybir.dt.float32

    nc = bacc.Bacc("TRN2", target_bir_lowering=False, debug=False,
                   num_devices=N_CORES)
    x = nc.dram_tensor("x", [H, IN_COLS], mm_dt, kind="ExternalInput")
    bands = nc.dram_tensor("bands", [128, KW, MT], mm_dt, kind="ExternalInput")
    biasb = nc.dram_tensor("biasb", [128, 1], f32, kind="ExternalInput")
    y = nc.dram_tensor("y", [OH, CPC], f32, kind="ExternalOutput")

    with tile.TileContext(nc) as tc:
        with (
            tc.tile_pool(name="const", bufs=1) as cpool,
            tc.tile_pool(name="xs", bufs=4) as xpool,
            tc.tile_pool(name="out", bufs=3) as opool,
            tc.tile_pool(name="ps", bufs=4, space="PSUM") as pspool,
        ):
            bands_t = cpool.tile([128, KW, MT], mm_dt)
            nc.sync.dma_start(bands_t[:], bands[:])
            bias_t = cpool.tile([128, 1], f32)
            nc.sync.dma_start(bias_t[:], biasb[:])

            for rep in range(repeat):
                for r0, emit0 in ROW_TILES:
                    xs = xpool.tile([128, IN_COLS], mm_dt, tag="xs")
                    # input rows r0..r0+128 are one contiguous block of the
                    # 518-wide slab; scalar-engine ring runs parallel to the
                    # sync-engine ring carrying const + output DMAs
                    nc.scalar.dma_start(xs[:], x[r0:r0 + 128, :])
                    outt = opool.tile([128, CPC], f32, tag="out")
                    ps = pspool.tile([128, CPC], f32, tag="ps")
                    for dx in range(KW):
                        nc.tensor.matmul(
                            ps[:MT, :],
                            bands_t[:, dx, :],
                            xs[:, dx: dx + CPC],
                            start=(dx == 0),
                            stop=(dx == KW - 1),
                        )
                    nc.vector.tensor_scalar_add(
                        outt[:MT, :], ps[:MT, :], bias_t[:MT, 0:1])
                    nc.sync.dma_start(
                        y[r0 + emit0: r0 + MT, :], outt[emit0:MT, :])

    nc.compile()
    return nc


def _get_program():
    if "nc" not in _cache:
        _cache["nc"] = _build_program()
    return _cache["nc"]


def _shard_inputs(X, weight, bias):
    X = np.ascontiguousarray(np.asarray(X, dtype=np.float32))
    weight = np.asarray(weight, dtype=np.float32)
    bias = np.asarray(bias, dtype=np.float32)

    # Host-side sharding: per-core column strip with halo cols (zero-padded at
    # the right edge for the last core; those output cols are discarded).
    slabs = np.zeros((N_CORES, H, IN_COLS), dtype=np.float32)
    for i in range(N_CORES):
        c0 = CPC * i
        c1 = min(c0 + IN_COLS, W)
        slabs[i, :, : c1 - c0] = X[:, c0:c1]

    # Banded-Toeplitz stationary matrices: bands[k, dx, m] = w[k-m, dx].
    bands = np.zeros((128, KW, MT), dtype=np.float32)
    for dy in range(KH):
        for m in range(MT):
            bands[m + dy, :, m] = weight[dy, :]

    biasb = np.broadcast_to(bias.reshape(1, 1), (128, 1)).copy()

    return [{"x": slabs[i], "bands": bands, "biasb": biasb}
            for i in range(N_CORES)]


def kernel(X, weight, bias):
    from concourse.bass_utils import run_bass_kernel_spmd

    nc = _get_program()
    in_maps = _shard_inputs(X, weight, bias)
    res = run_bass_kernel_spmd(nc, in_maps, list(range(N_CORES)))

    out = np.empty((OH, OW), dtype=np.float32)
    for i in range(N_CORES):
        c0 = CPC * i
        ncols = min(CPC, OW - c0)
        out[:, c0:c0 + ncols] = res.results[i]["y"][:, :ncols]
    return out


# revision 3
# speedup vs baseline: 1.7248x; 1.6378x over previous
"""Trainium2 Bass kernel for 7x7 valid cross-correlation on a 4096x4096 image.

Strategy: shard output COLUMNS across 8 NeuronCores (512 cols/core + 6 halo
input cols). PE matmul cost is proportional to the streamed free dim (width)
and independent of the output-row count M, so column strips let every core
run the global minimum ceil(4090/122)=34 band passes over a 1/8-width strip
(34 * 7 * 512 cycles ~= 50.8us/core), vs 5 full-width passes (59.7us) for
row sharding.

On-core, each band pass is 7 PSUM-accumulating matmuls: for kernel column dx,
a banded-Toeplitz stationary matrix B_dx[k, m] = w[k-m, dx] contracts over
128 input rows while the moving operand is the column-shifted view
xs[:, dx:dx+512] of the strip tile in SBUF. Every matmul is the same shape
(K=128, M=122, N=512); the last band pass overlaps its predecessor and only
emits the fresh rows.

All HBM traffic is bf16 (x, bands, y; PSUM accumulates fp32): with 8 cores
sharing the chip's HBM, fp32 traffic (135MB/iter) is what pushed measured
time past the PE roofline. bf16 halves it; the host converts y back to fp32.
bf16 matmul streams at the same 1 cycle/row as fp32r, and the 8-bit mantissa
keeps max error ~100x under the 2e-2 gate.

A warm-up block of 7 dummy matmuls on a memset tile runs while the first
input DMAs are in flight, so the PE's DVFS ramp (~3us to full clock) burns
DMA-latency time instead of slowing the first ~26 real matmuls.
"""

import numpy as np

H, W = 4096, 4096
KH, KW = 7, 7
N_CORES = 8
OH, OW = H - KH + 1, W - KW + 1          # 4090, 4090
CPC = 512                                 # output cols per core (core 7: 506 valid)
IN_COLS = CPC + KW - 1                    # 518 input cols per core (zero-padded for core 7)
MT = 122                                  # output rows per band pass (128 - 6)
# (input row offset, first fresh output row within the pass)
ROW_TILES = [(122 * t, 0) for t in range(33)] + [(H - 128, 58)]
N_WARMUP = 7                              # dummy matmuls to ramp the PE clock

_cache = {}


def _build_program(repeat=1):
    import concourse.bacc as bacc
    import concourse.mybir as mybir
    import concourse.tile as tile

    bf16 = mybir.dt.bfloat16
    f32 = mybir.dt.float32

    nc = bacc.Bacc("TRN2", target_bir_lowering=False, debug=False,
                   num_devices=N_CORES)
    x = nc.dram_tensor("x", [H, IN_COLS], bf16, kind="ExternalInput")
    bands = nc.dram_tensor("bands", [128, KW, MT], bf16, kind="ExternalInput")
    biasb = nc.dram_tensor("biasb", [128, 1], f32, kind="ExternalInput")
    y = nc.dram_tensor("y", [OH, CPC], bf16, kind="ExternalOutput")

    with tile.TileContext(nc) as tc:
        with (
            tc.tile_pool(name="const", bufs=1) as cpool,
            tc.tile_pool(name="xs", bufs=4) as xpool,
            tc.tile_pool(name="out", bufs=3) as opool,
            tc.tile_pool(name="ps", bufs=4, space="PSUM") as pspool,
        ):
            bands_t = cpool.tile([128, KW, MT], bf16)
            nc.sync.dma_start(bands_t[:], bands[:])
            bias_t = cpool.tile([128, 1], f32)
            nc.sync.dma_start(bias_t[:], biasb[:])

            # PE clock warm-up: no DMA dependency, so these run during the
            # first tiles' DMA latency and the ramp is over before real work.
            warm = cpool.tile([128, 512], bf16)
            nc.vector.memset(warm[:], 0)
            for i in range(N_WARMUP):
                wps = pspool.tile([128, 512], f32, tag="ps")
                nc.tensor.matmul(wps[:MT, :], warm[:, :MT], warm[:],
                                 start=True, stop=True)

            for rep in range(repeat):
                for r0, emit0 in ROW_TILES:
                    xs = xpool.tile([128, IN_COLS], bf16, tag="xs")
                    # input rows r0..r0+128 are one contiguous block of the
                    # 518-wide slab; scalar-engine ring runs parallel to the
                    # sync-engine ring carrying const + output DMAs
                    nc.scalar.dma_start(xs[:], x[r0:r0 + 128, :])
                    outt = opool.tile([128, CPC], bf16, tag="out")
                    ps = pspool.tile([128, CPC], f32, tag="ps")
                    for dx in range(KW):
                        nc.tensor.matmul(
                            ps[:MT, :],
                            bands_t[:, dx, :],
                            xs[:, dx: dx + CPC],
                            start=(dx == 0),
                            stop=(dx == KW - 1),
                        )
                    nc.vector.tensor_scalar_add(
                        outt[:MT, :], ps[:MT, :], bias_t[:MT, 0:1])
                    nc.sync.dma_start(
                        y[r0 + emit0: r0 + MT, :], outt[emit0:MT, :])

    nc.compile()
    return nc


def _get_program():
    if "nc" not in _cache:
        _cache["nc"] = _build_program()
    return _cache["nc"]


def _shard_inputs(X, weight, bias):
    import ml_dtypes

    bf16 = ml_dtypes.bfloat16
    X = np.asarray(X, dtype=np.float32)
    weight = np.asarray(weight, dtype=np.float32)
    bias = np.asarray(bias, dtype=np.float32)

    # Host-side sharding: per-core column strip with halo cols (zero-padded at
    # the right edge for the last core; those output cols are discarded).
    slabs = np.zeros((N_CORES, H, IN_COLS), dtype=bf16)
    for i in range(N_CORES):
        c0 = CPC * i
        c1 = min(c0 + IN_COLS, W)
        slabs[i, :, : c1 - c0] = X[:, c0:c1]

    # Banded-Toeplitz stationary matrices: bands[k, dx, m] = w[k-m, dx].
    bands = np.zeros((128, KW, MT), dtype=np.float32)
    for dy in range(KH):
        for m in range(MT):
            bands[m + dy, :, m] = weight[dy, :]
    bands = bands.astype(bf16)

    biasb = np.broadcast_to(bias.reshape(1, 1), (128, 1)).copy()

    return [{"x": slabs[i], "bands": bands, "biasb": biasb}
            for i in range(N_CORES)]


def kernel(X, weight, bias):
    from concourse.bass_utils import run_bass_kernel_spmd

    nc = _get_program()
    in_maps = _shard_inputs(X, weight, bias)
    res = run_bass_kernel_spmd(nc, in_maps, list(range(N_CORES)))

    out = np.empty((OH, OW), dtype=np.float32)
    for i in range(N_CORES):
        c0 = CPC * i
        ncols = min(CPC, OW - c0)
        out[:, c0:c0 + ncols] = res.results[i]["y"][:, :ncols].astype(
            np.float32)
    return out


# revision 6
# speedup vs baseline: 2.1751x; 1.2610x over previous
"""Trainium2 Bass kernel for 7x7 valid cross-correlation on a 4096x4096 image.

Strategy: shard output COLUMNS across 8 NeuronCores (512 cols/core + 6 halo
input cols). PE matmul cost is proportional to the streamed free dim (width)
and independent of the output-row count M, so column strips let every core
run the global minimum ceil(4090/122)=34 band passes over a 1/8-width strip
(34 * 7 * 512 cycles ~= 50.8us/core), vs 5 full-width passes (59.7us) for
row sharding.

On-core, each band pass is 7 PSUM-accumulating matmuls: for kernel column dx,
a banded-Toeplitz stationary matrix B_dx[k, m] = w[k-m, dx] contracts over
128 input rows while the moving operand is the column-shifted view
xs[:, dx:dx+512] of the strip tile in SBUF. Every matmul is the same shape
(K=128, M=122, N=512); the last band pass overlaps its predecessor and only
emits the fresh rows.

All HBM traffic is bf16 (x, bands, y; PSUM accumulates fp32): with 8 cores
sharing the chip's HBM, fp32 traffic (135MB/iter) is what pushed measured
time past the PE roofline. bf16 halves it; the host converts y back to fp32.
bf16 matmul streams at the same 1 cycle/row as fp32r, and the 8-bit mantissa
keeps max error ~100x under the 2e-2 gate.

A warm-up block of 7 dummy matmuls on a memset tile runs while the first
input DMAs are in flight, so the PE's DVFS ramp (~3us to full clock) burns
DMA-latency time instead of slowing the first ~26 real matmuls.
"""

import numpy as np

H, W = 4096, 4096
KH, KW = 7, 7
N_CORES = 8
OH, OW = H - KH + 1, W - KW + 1          # 4090, 4090
CPC = 512                                 # output cols per core (core 7: 506 valid)
IN_COLS = CPC + KW - 1                    # 518 input cols per core (zero-padded for core 7)
MT = 122                                  # output rows per band pass (128 - 6)
# (input row offset, first fresh output row within the pass)
ROW_TILES = [(122 * t, 0) for t in range(33)] + [(H - 128, 58)]
N_WARMUP = 5                              # dummy matmuls to ramp the PE clock

_cache = {}


def _build_program(repeat=1):
    import concourse.bacc as bacc
    import concourse.mybir as mybir
    import concourse.tile as tile

    bf16 = mybir.dt.bfloat16
    f32 = mybir.dt.float32

    nc = bacc.Bacc("TRN2", target_bir_lowering=False, debug=False,
                   num_devices=N_CORES)
    x = nc.dram_tensor("x", [H, IN_COLS], bf16, kind="ExternalInput")
    bands = nc.dram_tensor("bands", [128, KW, MT], bf16, kind="ExternalInput")
    biasb = nc.dram_tensor("biasb", [128, 1], f32, kind="ExternalInput")
    y = nc.dram_tensor("y", [OH, CPC], bf16, kind="ExternalOutput")

    with tile.TileContext(nc) as tc:
        with (
            tc.tile_pool(name="const", bufs=1) as cpool,
            tc.tile_pool(name="xs", bufs=4) as xpool,
            tc.tile_pool(name="out", bufs=3) as opool,
            tc.tile_pool(name="ps", bufs=4, space="PSUM") as pspool,
        ):
            bands_t = cpool.tile([128, KW, MT], bf16)
            nc.sync.dma_start(bands_t[:], bands[:])
            bias_t = cpool.tile([128, 1], f32)
            nc.sync.dma_start(bias_t[:], biasb[:])

            # PE clock warm-up: no DMA dependency, so these run during the
            # first tiles' DMA latency and the ramp is over before real work.
            warm = cpool.tile([128, 512], bf16)
            nc.vector.memset(warm[:], 0)
            for i in range(N_WARMUP):
                wps = pspool.tile([128, 512], f32, tag="ps")
                nc.tensor.matmul(wps[:MT, :], warm[:, :MT], warm[:],
                                 start=True, stop=True)

            for rep in range(repeat):
                for r0, emit0 in ROW_TILES:
                    xs = xpool.tile([128, IN_COLS], bf16, tag="xs")
                    # input rows r0..r0+128 are one contiguous block of the
                    # 518-wide slab; scalar-engine ring runs parallel to the
                    # sync-engine ring carrying const + output DMAs
                    nc.scalar.dma_start(xs[:], x[r0:r0 + 128, :])
                    outt = opool.tile([128, CPC], bf16, tag="out")
                    ps = pspool.tile([128, CPC], f32, tag="ps")
                    for dx in range(KW):
                        nc.tensor.matmul(
                            ps[:MT, :],
                            bands_t[:, dx, :],
                            xs[:, dx: dx + CPC],
                            start=(dx == 0),
                            stop=(dx == KW - 1),
                        )
                    nc.vector.tensor_scalar_add(
                        outt[:MT, :], ps[:MT, :], bias_t[:MT, 0:1])
                    nc.sync.dma_start(
                        y[r0 + emit0: r0 + MT, :], outt[emit0:MT, :])

    nc.compile()
    return nc


def _get_program():
    if "nc" not in _cache:
        _cache["nc"] = _build_program()
    return _cache["nc"]


def _shard_inputs(X, weight, bias):
    import ml_dtypes

    bf16 = ml_dtypes.bfloat16
    X = np.asarray(X, dtype=np.float32)
    weight = np.asarray(weight, dtype=np.float32)
    bias = np.asarray(bias, dtype=np.float32)

    # Host-side sharding: per-core column strip with halo cols (zero-padded at
    # the right edge for the last core; those output cols are discarded).
    slabs = np.zeros((N_CORES, H, IN_COLS), dtype=bf16)
    for i in range(N_CORES):
        c0 = CPC * i
        c1 = min(c0 + IN_COLS, W)
        slabs[i, :, : c1 - c0] = X[:, c0:c1]

    # Banded-Toeplitz stationary matrices: bands[k, dx, m] = w[k-m, dx].
    bands = np.zeros((128, KW, MT), dtype=np.float32)
    for dy in range(KH):
        for m in range(MT):
            bands[m + dy, :, m] = weight[dy, :]
    bands = bands.astype(bf16)

    biasb = np.broadcast_to(bias.reshape(1, 1), (128, 1)).copy()

    return [{"x": slabs[i], "bands": bands, "biasb": biasb}
            for i in range(N_CORES)]


def kernel(X, weight, bias):
    from concourse.bass_utils import run_bass_kernel_spmd

    nc = _get_program()
    in_maps = _shard_inputs(X, weight, bias)
    res = run_bass_kernel_spmd(nc, in_maps, list(range(N_CORES)))

    out = np.empty((OH, OW), dtype=np.float32)
    for i in range(N_CORES):
        c0 = CPC * i
        ncols = min(CPC, OW - c0)
        out[:, c0:c0 + ncols] = res.results[i]["y"][:, :ncols].astype(
            np.float32)
    return out
